# revision 42
# baseline (speedup 1.0000x reference)
"""GQA attention (RoPE + ALiBi + causal) Bass kernel for Trainium2, 8 NeuronCores.

Sharding: core (b, g) = batch b in {0,1} x kv-group g in {0..3}; each core computes
its 4 query heads' attention for its batch and a partial output projection
(row-parallel wo); host sums the 4 group partials per batch.

Dataflow is 16-bit on the PE: fp16 for x/weights/Q/K/attn, bf16 for P and V
(bf16's fp32-size exponent carries the ALiBi recentering range).

  Phase 1 (per 512-q window): Q/K/V projections (6 PSUM accumulators over one
  streamed pass of xT). Weights are host-packed into the [128, ND, out] SBUF
  layout so their DMAs are wide contiguous lines; the first chunks plus
  window-0 x go on the sync hwdge queue, later chunks stream from gpsimd.
  x tiles for window w+1 are prefetched during window w (one step of
  lookahead per d). PSUM->SBUF copies split across ACT/DVE, per-window RoPE
  (DVE) and V transpose (PE). Window 3's rope is DEFERRED into early phase 2
  (its consumers are the w=3 units at the very end) so the DVE backlog at the
  phase boundary doesn't stall the first attention units.
  Phase 2: per (window, head): scoresT = K^T Q (PSUM), P = exp(scale*scores
  + bias) with per-head-slot exp widths {128,256,256,512}: heads are ordered by
  descending ALiBi slope within each GQA group, so later slots (smaller slopes)
  tolerate coarser per-chunk bias recentering - fewer, wider ACT instructions.
  The -slope*q half of ALiBi cancels in softmax; the per-kv half plus the
  chunk-recentering constant comes from a host-built bias table indexed by
  (slot, chunkpos - kvtile). Diagonal chunks get a causal 0/1 mask multiply.
  Then outT += V_u^T P and den += ones^T P (PSUM accumulate); attn =
  outT * recip(den); each window's output-projection matmuls are interleaved
  as PE filler into the next window's attention loop, front-loaded onto the
  exp-heavy slot-0 head. Window 0 runs its heads cheapest-exp-first while
  the filler queue is empty.
  The final drain alternates staging copies across DVE/ACT and rotates the
  output DMAs over the sync/gpsimd/scalar queues.
"""
import math
from contextlib import ExitStack

import numpy as np
import ml_dtypes

import concourse.bass as bass
import concourse.bacc as bacc
import concourse.tile as tile
from concourse import mybir
from concourse.bass_utils import run_bass_kernel_spmd

F32 = mybir.dt.float32
F16 = mybir.dt.float16
BF16 = mybir.dt.bfloat16

B, S, D = 2, 2048, 2048
H, KV, HD, REP = 16, 4, 128, 4
NH = 4                     # heads per core
NW = S // 512              # q-windows
ND = D // 128              # d_in tiles
NU = S // 128              # kv tiles
SCALE = 1.0 / math.sqrt(HD)

# per-head-slot exp chunk width and bias recentering constant; slot 0 holds the
# steepest ALiBi slope of the core's group (heads are slope-descending within a
# group), so it gets the finest recentering. Range check (worst slope s per
# slot, scores*scale ~ N(0,1), P in bf16 with fp32-range exponent):
#   exp arg in [score - s*C, score + s*(W-1-C)] -> within e^{+-87} for
#   (W,C,s) = (128,96,.707), (256,127,.5), (256,127,.354), (512,255,.25).
EXP_W = [128, 256, 256, 512]
EXP_C = [96.0, 127.0, 127.0, 255.0]
NBM = 19                   # bias cols per head slot: m = gridpos - kvtile in [-3, 15]
# rope-pair swap within each 32-partition quadrant (pairs live at +0/+16)
SHUF_MASK = list(range(16, 32)) + list(range(16))


def build():
    nc = bacc.Bacc(None)
    xT_d = nc.dram_tensor("xT", [D, S], F16, kind="ExternalInput")
    # weights host-packed as [128, ND, out]: DMA lines are contiguous per
    # partition row (2KB+ for wq), full DMA throughput from the first chunk
    wq_d = nc.dram_tensor("wqT", [128, ND, NH * HD], F16, kind="ExternalInput")
    wk_d = nc.dram_tensor("wkT", [128, ND, HD], F16, kind="ExternalInput")
    wv_d = nc.dram_tensor("wvT", [128, ND, HD], F16, kind="ExternalInput")
    wo_d = nc.dram_tensor("woT", [NH * HD, D], F16, kind="ExternalInput")
    cosF_d = nc.dram_tensor("cosF", [128, S], F16, kind="ExternalInput")
    sinF_d = nc.dram_tensor("sinF", [128, S], F16, kind="ExternalInput")
    biasb_d = nc.dram_tensor("biasb", [128, NH * NBM], F32, kind="ExternalInput")
    cmask_d = nc.dram_tensor("cmask", [128, 128], BF16, kind="ExternalInput")
    ident_d = nc.dram_tensor("ident", [128, 128], BF16, kind="ExternalInput")
    ones_d = nc.dram_tensor("ones", [128, 128], BF16, kind="ExternalInput")
    part_d = nc.dram_tensor("part", [S, D], F16, kind="ExternalOutput")

    PSUM = bass.MemorySpace.PSUM

    with tile.TileContext(nc) as tc:
        with ExitStack() as ctx:
            consts = ctx.enter_context(tc.tile_pool(name="consts", bufs=1))
            persist = ctx.enter_context(tc.tile_pool(name="persist", bufs=1))

            cosF = consts.tile([128, S], F16, tag="cosF")
            sinF = consts.tile([128, S], F16, tag="sinF")
            biasb = consts.tile([128, NH * NBM], F32, tag="biasb")
            cmask = consts.tile([128, 128], BF16, tag="cmask")
            ident = consts.tile([128, 128], BF16, tag="ident")
            ones = consts.tile([128, 128], BF16, tag="ones")

            qT = [persist.tile([128, S], F16, tag=f"qT{h}", name=f"qT{h}")
                  for h in range(NH)]
            kT = persist.tile([128, S], F16, tag="kT")
            vnat = persist.tile([128, S], BF16, tag="vnat")
            attn = [persist.tile([128, S], F16, tag=f"attn{h}", name=f"attn{h}")
                    for h in range(NH)]

            # ---------------- phase 1: Q/K/V projections (+ per-window RoPE) -----
            with tc.tile_pool(name="wqkv", bufs=1) as wpool, \
                 tc.tile_pool(name="xsl", bufs=14) as xpool, \
                 tc.tile_pool(name="vtmp", bufs=1) as vpool, \
                 tc.tile_pool(name="rope", bufs=3) as rp, \
                 tc.tile_pool(name="pps", bufs=1, space=PSUM) as pps, \
                 tc.tile_pool(name="tpp", bufs=1, space=PSUM) as tpp:
                wq_sb = wpool.tile([128, ND, NH * HD], F16, tag="wq")
                wk_sb = wpool.tile([128, ND, HD], F16, tag="wk")
                wv_sb = wpool.tile([128, ND, HD], F16, tag="wv")

                xtiles = {}

                def issue_x(w, d):
                    xs = xpool.tile([128, 512], F16, tag="x", name="xs")
                    nc.sync.dma_start(
                        xs[:], xT_d[d * 128:(d + 1) * 128, w * 512:(w + 1) * 512])
                    xtiles[(w, d)] = xs

                # critical path on the sync hwdge queue (first to issue):
                # the small wk/wv head chunks, then window-0 x tiles;
                # remaining weight chunks stream JIT from gpsimd, followed by
                # the rope tables/consts (first needed ~25us in)
                d01 = slice(0, 2)
                nc.sync.dma_start(wk_sb[:, d01, :], wk_d[:, d01, :])
                nc.sync.dma_start(wv_sb[:, d01, :], wv_d[:, d01, :])
                issue_x(0, 0)
                nc.sync.dma_start(wq_sb[:, 0:1, :], wq_d[:, 0:1, :])
                nc.sync.dma_start(wq_sb[:, 1:2, :], wq_d[:, 1:2, :])
                for g2 in range(1, ND // 2):
                    dsl = slice(2 * g2, 2 * g2 + 2)
                    nc.gpsimd.dma_start(wk_sb[:, dsl, :], wk_d[:, dsl, :])
                    nc.gpsimd.dma_start(wv_sb[:, dsl, :], wv_d[:, dsl, :])
                    nc.gpsimd.dma_start(wq_sb[:, dsl, :], wq_d[:, dsl, :])
                for d in range(1, ND):
                    issue_x(0, d)
                nc.gpsimd.dma_start(cosF[:], cosF_d[:])
                nc.gpsimd.dma_start(sinF[:], sinF_d[:])
                nc.gpsimd.dma_start(biasb[:], biasb_d[:])
                nc.gpsimd.dma_start(cmask[:], cmask_d[:])
                nc.gpsimd.dma_start(ident[:], ident_d[:])
                nc.gpsimd.dma_start(ones[:], ones_d[:])
                vT = vpool.tile([128, S], BF16, tag="vT")

                for w in range(NW):
                    sl = slice(w * 512, (w + 1) * 512)
                    pq = [pps.tile([128, 512], F32, tag=f"pq{h}", name=f"pq{h}")
                          for h in range(NH)]
                    pk = pps.tile([128, 512], F32, tag="pk", name="pk")
                    pv = pps.tile([128, 512], F32, tag="pv", name="pv")
                    for d in range(ND):
                        if w < NW - 1:
                            issue_x(w + 1, d)   # one-window lookahead
                        xs = xtiles.pop((w, d))
                        st, sp = (d == 0), (d == ND - 1)
                        nc.tensor.matmul(pk[:], wk_sb[:, d, :], xs[:], start=st, stop=sp)
                        nc.tensor.matmul(pv[:], wv_sb[:, d, :], xs[:], start=st, stop=sp)
                        for h in range(NH):
                            nc.tensor.matmul(pq[h][:], wq_sb[:, d, h * 128:(h + 1) * 128],
                                             xs[:], start=st, stop=sp)
                    # PSUM->SBUF copies split across ACT and DVE
                    nc.scalar.copy(kT[:, sl], pk[:])
                    nc.scalar.copy(qT[0][:, sl], pq[0][:])
                    nc.scalar.copy(qT[1][:, sl], pq[1][:])
                    nc.vector.tensor_copy(vT[:, sl], pv[:])
                    nc.vector.tensor_copy(qT[2][:, sl], pq[2][:])
                    nc.vector.tensor_copy(qT[3][:, sl], pq[3][:])

                    # RoPE on this window's q/k slices: out = cosF*z +
                    # sinF*swap(z); rope pairs are laid out within 32-partition
                    # quadrants (host perm) so the swap is a DVE stream_shuffle.
                    # The last window's rope is deferred into phase 2 (its
                    # consumers are the w=3 attention units).
                    if w < NW - 1:
                        for tgt in [kT] + qT:
                            qb = rp.tile([128, 512], F16, tag="qb", name="qb")
                            nc.vector.stream_shuffle(qb[:], tgt[:, sl], SHUF_MASK)
                            t1 = rp.tile([128, 512], F16, tag="t1", name="t1")
                            nc.vector.tensor_mul(t1[:], tgt[:, sl], cosF[:, sl])
                            nc.vector.tensor_mul(qb[:], qb[:], sinF[:, sl])
                            nc.vector.tensor_add(tgt[:, sl], t1[:], qb[:])

                    # V transpose for this window's 4 kv tiles:
                    # vT [hd, s] -> vnat [s(part), hd]
                    for u in range(4 * w, 4 * w + 4):
                        tp = tpp.tile([128, 128], BF16, tag=f"tp{u % 2}",
                                      name=f"tp{u}")
                        nc.tensor.transpose(tp[:], vT[:, u * 128:(u + 1) * 128],
                                            ident[:])
                        nc.scalar.copy(vnat[:, u * 128:(u + 1) * 128], tp[:])

            # ---------------- phase 2: attention + output projection ------------
            with tc.tile_pool(name="sp", bufs=3, space=PSUM) as sp, \
                 tc.tile_pool(name="dp", bufs=1, space=PSUM) as dp, \
                 tc.tile_pool(name="op", bufs=2, space=PSUM) as op, \
                 tc.tile_pool(name="ojp", bufs=2, space=PSUM) as ojp, \
                 tc.tile_pool(name="Pp", bufs=8) as Pp, \
                 tc.tile_pool(name="psum_sb", bufs=2) as psb, \
                 tc.tile_pool(name="rope2", bufs=2) as rp2, \
                 tc.tile_pool(name="ep", bufs=4) as ep, \
                 tc.tile_pool(name="wop", bufs=1) as wop, \
                 tc.tile_pool(name="ostg", bufs=6) as ostg:
                wo_sb = wop.tile([128, NH, D], F16, tag="wo")
                nc.gpsimd.dma_start(wo_sb[:], wo_d.rearrange("(h p) o -> p h o", p=128))

                # deferred rope for the last window, spread one tensor per
                # early phase-2 unit (consumers are the w=3 units much later)
                sl3 = slice((NW - 1) * 512, NW * 512)
                rope_q = [qT[3], qT[2], qT[1], qT[0], kT]

                def emit_rope():
                    if not rope_q:
                        return
                    tgt = rope_q.pop(0)
                    qb = rp2.tile([128, 512], F16, tag="qb2", name="qb2")
                    nc.vector.stream_shuffle(qb[:], tgt[:, sl3], SHUF_MASK)
                    t1 = rp2.tile([128, 512], F16, tag="t12", name="t12")
                    nc.vector.tensor_mul(t1[:], tgt[:, sl3], cosF[:, sl3])
                    nc.vector.tensor_mul(qb[:], qb[:], sinF[:, sl3])
                    nc.vector.tensor_add(tgt[:, sl3], t1[:], qb[:])

                filler_q = []

                def emit_fillers(n, eng="v", queue=None):
                    for _ in range(n):
                        if not filler_q:
                            return
                        filler_q.pop(0)(eng, queue)

                def make_unit(w_, mq_, dwin_):
                    def unit(eng, queue):
                        m_ = 4 * w_ + mq_
                        po = ojp.tile([128, 512], F32, tag="oj",
                                      name=f"po{m_}_{dwin_}")
                        for h_ in range(NH):
                            nc.tensor.matmul(
                                po[:],
                                attn[h_][:, m_ * 128:(m_ + 1) * 128],
                                wo_sb[:, h_, dwin_ * 512:(dwin_ + 1) * 512],
                                start=(h_ == 0), stop=(h_ == NH - 1))
                        so = ostg.tile([128, 512], F16, tag="so", name="so")
                        if eng == "v":
                            nc.vector.tensor_copy(so[:], po[:])
                        else:
                            nc.scalar.copy(so[:], po[:])
                        dq_eng = queue if queue is not None else nc.sync
                        dq_eng.dma_start(
                            part_d[m_ * 128:(m_ + 1) * 128,
                                   dwin_ * 512:(dwin_ + 1) * 512], so[:])
                    return unit

                ucount = 0
                # per-window filler emission caps: the late windows are
                # ACT/DVE-paced with PE slack, the early ones are PE-paced;
                # carrying units from w=1 into w=2/3 matches filler supply
                # to where the PE actually idles (trace: filler_q exhausted
                # halfway through w=2/3 leaving ~0.5us gaps every 2 u-steps)
                emit_cap = {0: 0, 1: 8, 2: 16, 3: 22}
                for w in range(NW):
                    emitted_in_w = 0
                    qsl = slice(w * 512, (w + 1) * 512)
                    U = 4 * (w + 1)
                    # window 0 runs cheapest-exp heads first (no filler supply
                    # yet, so let ACT race ahead of the PE)
                    horder = [3, 2, 1, 0] if w == 0 else [0, 1, 2, 3]
                    for h in horder:
                        o_ps = op.tile([128, 512], F32, tag="o", name=f"o{w}_{h}")
                        d_ps = dp.tile([128, 512], F32, tag="den", name=f"d{w}_{h}")
                        # hybrid softmax denominator: for w>=1 the early full
                        # tiles (u <= U-4, all n0=0) accumulate elementwise
                        # into a bf16 partial sum on the DVE, reduced by ONE
                        # ones^T matmul; only the last 3 diagonal tiles keep
                        # the per-tile PE matmul path (their adds would land
                        # too late to hide). This removes ~2/3 of the den's
                        # PE column traffic. w=0 units are tiny: all-PE.
                        chain = w >= 1
                        psA = psb.tile([128, 512], BF16, tag="psA",
                                       name="psA") if chain else None
                        # PV runs LAG tiles behind the scores/exp front so
                        # each exp has a full extra iteration to land before
                        # the PE consumes its P tile (the scores->exp->PV
                        # round trip was showing ~0.5us PE waits per 2 steps
                        # in the late windows)
                        lag = 2 if chain else 1
                        pendq = []
                        dq = []
                        for u in range(U):
                            i0 = max(0, u - 4 * w)
                            n0 = 128 * i0
                            s_ps = sp.tile([128, 512], F32, tag="s", name="s")
                            nc.tensor.matmul(
                                s_ps[:, n0:512],
                                kT[:, u * 128:(u + 1) * 128],
                                qT[h][:, w * 512 + n0:(w + 1) * 512],
                                start=True, stop=True)
                            if chain and u == U - 1:
                                # partial-sum reduce; its last add (u=U-4)
                                # finished iterations ago, so no PE stall
                                nc.tensor.matmul(d_ps[:], ones[:], psA[:],
                                                 start=True, stop=False)
                            Pt = Pp.tile([128, 512], BF16, tag="P", name="P")
                            # exp pieces on a fixed per-slot q-grid (grid
                            # position independent of u so the dropped per-q
                            # ALiBi term is consistent across kv tiles);
                            # diagonal tiles clip the first piece at n0.
                            # bias col m = grid position (128-units) - kv tile
                            for gs in range(0, 512, EXP_W[h]):
                                pcs = max(gs, n0)
                                pce = gs + EXP_W[h]
                                if pcs >= pce:
                                    continue
                                m = 4 * w + gs // 128 - u + 3
                                nc.scalar.activation(
                                    Pt[:, pcs:pce],
                                    s_ps[:, pcs:pce],
                                    mybir.ActivationFunctionType.Exp,
                                    bias=biasb[:, h * NBM + m:h * NBM + m + 1],
                                    scale=SCALE)
                            if u >= 4 * w:
                                # diagonal kv tile: causal 0/1 mask on its block
                                nc.vector.tensor_mul(Pt[:, n0:n0 + 128],
                                                     Pt[:, n0:n0 + 128], cmask[:])
                            if chain and u <= U - 4:
                                if u == 0:
                                    nc.vector.tensor_copy(psA[:], Pt[:])
                                else:
                                    nc.vector.tensor_add(psA[:], psA[:], Pt[:])
                            ucount += 1
                            # front-load PE filler onto the exp-heavy slot-0
                            # head; later slots are self-paced. Hold fillers
                            # for the first u-steps of each window's first
                            # head: they read the PREVIOUS window's attn whose
                            # recip/mul chain is still draining on DVE, and
                            # they would block the new window's scores in the
                            # in-order PE queue.
                            if ((h == 0 and u >= 2)
                                    or (h != 0 and ucount % 2 == 0)) \
                                    and emitted_in_w < emit_cap[w]:
                                emit_fillers(1)
                                emitted_in_w += 1
                            pendq.append((Pt, n0, u))
                            if len(pendq) > lag:
                                pPt, pn0, pu = pendq.pop(0)
                                nc.tensor.matmul(o_ps[:, pn0:512],
                                                 vnat[:, pu * 128:(pu + 1) * 128],
                                                 pPt[:, pn0:512],
                                                 start=(pu == 0), stop=False)
                                if not chain or pu >= U - 3:
                                    dq.append((pPt, pn0, pu))
                                if len(dq) > 2 and not chain:
                                    dPt, dn0, du = dq.pop(0)
                                    nc.tensor.matmul(d_ps[:, dn0:512], ones[:],
                                                     dPt[:, dn0:512],
                                                     start=(du == 0), stop=False)
                        for pPt, pn0, pu in pendq:
                            nc.tensor.matmul(o_ps[:, pn0:512],
                                             vnat[:, pu * 128:(pu + 1) * 128],
                                             pPt[:, pn0:512],
                                             start=(pu == 0), stop=(pu == U - 1))
                            if not chain or pu >= U - 3:
                                dq.append((pPt, pn0, pu))
                        pendq = []
                        for qi, (dPt, dn0, du) in enumerate(dq):
                            nc.tensor.matmul(d_ps[:, dn0:512], ones[:],
                                             dPt[:, dn0:512],
                                             start=(False if chain else du == 0),
                                             stop=(qi == len(dq) - 1))
                        dq = []
                        rec = ep.tile([128, 512], F32, tag="rec", name="rec")
                        nc.vector.reciprocal_approx_fast(rec[:], d_ps[:])
                        nc.vector.tensor_mul(attn[h][:, qsl], o_ps[:], rec[:])
                        # deferred window-3 rope rides the early-unit DVE slack
                        if w <= 1:
                            emit_rope()

                    # enqueue this window's output projection as PE filler
                    # for the next window's attention loop
                    for mq in range(4):
                        for dwin in range(4):
                            filler_q.append(make_unit(w, mq, dwin))

                # drain remaining units; staging copies alternate DVE/ACT and
                # the output DMAs rotate over four queues (all near-idle now)
                di = 0
                queues = [nc.sync, nc.gpsimd, nc.scalar]
                while filler_q:
                    eng = "v" if di % 2 == 0 else "s"
                    emit_fillers(1, eng=eng, queue=queues[di % 3])
                    di += 1
    nc.finalize()
    return nc


_NC_CACHE = {}


def _get_nc():
    if "nc" not in _NC_CACHE:
        _NC_CACHE["nc"] = build()
    return _NC_CACHE["nc"]


def _host_prep(x, alibi_bias, wq, wk, wv, wo):
    """Build per-core input maps (shard + transpose + rope tables + bias tables)."""
    x = np.asarray(x, np.float32)
    alibi_bias = np.asarray(alibi_bias, np.float32)
    wq = np.asarray(wq, np.float32)
    wk = np.asarray(wk, np.float32)
    wv = np.asarray(wv, np.float32)
    wo = np.asarray(wo, np.float32)
    bf16 = ml_dtypes.bfloat16

    slopes = alibi_bias[0, :, 0, 1].copy()        # [H]; alibi[0,h,0,1] = slope_h

    inv_freq = 1.0 / (10000.0 ** (np.arange(0, HD, 2, dtype=np.float32) / HD))
    t = np.arange(S, dtype=np.float32)
    freqs = np.outer(t, inv_freq)                 # [S, 64]
    cos = np.cos(freqs).astype(np.float32).T      # [64, S]
    sin = np.sin(freqs).astype(np.float32).T
    # quadrant-paired rope layout: pair i -> quadrant i//16, slots j and j+16
    # (even element at row 32*(i//16)+i%16, odd at +16); the on-device swap is
    # then a within-quadrant stream_shuffle
    perm = np.zeros(HD, np.int64)
    row_f = np.zeros(HD, np.int64)    # freq index per row
    row_sg = np.zeros(HD, np.float32)  # sin sign per row
    for i in range(64):
        qd, j = divmod(i, 16)
        perm[32 * qd + j] = 2 * i
        perm[32 * qd + 16 + j] = 2 * i + 1
        row_f[32 * qd + j] = i
        row_f[32 * qd + 16 + j] = i
        row_sg[32 * qd + j] = -1.0
        row_sg[32 * qd + 16 + j] = 1.0
    cosF = np.ascontiguousarray(cos[row_f]).astype(np.float16)
    sinF = np.ascontiguousarray(sin[row_f] * row_sg[:, None]).astype(np.float16)
    p_ar = np.arange(128, dtype=np.float32)
    cmask = (p_ar[:, None] <= p_ar[None, :]).astype(bf16)
    ident = np.eye(128, dtype=np.float32).astype(bf16)
    ones = np.ones((128, 128), np.float32).astype(bf16)

    def pack_ptd(wT):
        # [D, out] -> [128, ND, out] so SBUF-layout DMA lines are contiguous
        return np.ascontiguousarray(
            wT.reshape(ND, 128, wT.shape[1]).transpose(1, 0, 2))

    xTs = [np.ascontiguousarray(x[b].T).astype(np.float16) for b in range(B)]
    in_maps = []
    for core in range(8):
        b, g = divmod(core, KV)
        wq_g = wq[4 * g * HD:(4 * g + 4) * HD].reshape(NH, HD, D)[:, perm, :]
        wqT = np.ascontiguousarray(wq_g.reshape(NH * HD, D).T).astype(np.float16)
        wkT = np.ascontiguousarray(wk[g * HD:(g + 1) * HD][perm].T).astype(np.float16)
        wvT = np.ascontiguousarray(wv[g * HD:(g + 1) * HD].T).astype(np.float16)
        woT = np.ascontiguousarray(
            wo[:, 4 * g * HD:(4 * g + 4) * HD].T).astype(np.float16)
        biasb = np.zeros((128, NH * NBM), np.float32)
        for h in range(NH):
            sl = slopes[4 * g + h]
            for mi in range(NBM):
                m = mi - 3   # m = gridpos - kvtile; bias = slope*(j - i_gs - C)
                biasb[:, h * NBM + mi] = sl * (p_ar - 128.0 * m - EXP_C[h])
        in_maps.append({
            "xT": xTs[b], "wqT": pack_ptd(wqT), "wkT": pack_ptd(wkT),
            "wvT": pack_ptd(wvT), "woT": woT,
            "cosF": cosF, "sinF": sinF, "biasb": biasb, "cmask": cmask,
            "ident": ident, "ones": ones,
        })
    return in_maps


def kernel(x, mask, alibi_bias, wq, wk, wv, wo, _trace=False, _trace_kwargs=None):
    nc = _get_nc()
    in_maps = _host_prep(x, alibi_bias, wq, wk, wv, wo)
    res = run_bass_kernel_spmd(nc, in_maps, list(range(8)), trace=_trace,
                               **(_trace_kwargs or {}))
    parts = [np.asarray(res.results[c]["part"], np.float32) for c in range(8)]
    out = np.stack([
        parts[0] + parts[1] + parts[2] + parts[3],
        parts[4] + parts[5] + parts[6] + parts[7],
    ]).astype(np.float32)
    if _trace:
        return out, res
    return out


# revision 43
# speedup vs baseline: 1.0116x; 1.0116x over previous
"""GQA attention (RoPE + ALiBi + causal) Bass kernel for Trainium2, 8 NeuronCores.

Sharding: core (b, g) = batch b in {0,1} x kv-group g in {0..3}; each core computes
its 4 query heads' attention for its batch and a partial output projection
(row-parallel wo); host sums the 4 group partials per batch.

Dataflow is 16-bit on the PE: fp16 for x/weights/Q/K/attn, bf16 for P and V
(bf16's fp32-size exponent carries the ALiBi recentering range).

  Phase 1 (per 512-q window): Q/K/V projections (6 PSUM accumulators over one
  streamed pass of xT). Weights are host-packed into the [128, ND, out] SBUF
  layout so their DMAs are wide contiguous lines; the first chunks plus
  window-0 x go on the sync hwdge queue, later chunks stream from gpsimd.
  x tiles for window w+1 are prefetched during window w (one step of
  lookahead per d). PSUM->SBUF copies split across ACT/DVE, per-window RoPE
  (DVE) and V transpose (PE). Window 3's rope is DEFERRED into early phase 2
  (its consumers are the w=3 units at the very end) so the DVE backlog at the
  phase boundary doesn't stall the first attention units.
  Phase 2: per (window, head): scoresT = K^T Q (PSUM), P = exp(scale*scores
  + bias) with per-head-slot exp widths {128,256,256,512}: heads are ordered by
  descending ALiBi slope within each GQA group, so later slots (smaller slopes)
  tolerate coarser per-chunk bias recentering - fewer, wider ACT instructions.
  The -slope*q half of ALiBi cancels in softmax; the per-kv half plus the
  chunk-recentering constant comes from a host-built bias table indexed by
  (slot, chunkpos - kvtile). Diagonal chunks get a causal 0/1 mask multiply.
  Then outT += V_u^T P and den += ones^T P (PSUM accumulate); attn =
  outT * recip(den); each window's output-projection matmuls are interleaved
  as PE filler into the next window's attention loop, front-loaded onto the
  exp-heavy slot-0 head. Window 0 runs its heads cheapest-exp-first while
  the filler queue is empty.
  The final drain alternates staging copies across DVE/ACT and rotates the
  output DMAs over the sync/gpsimd/scalar queues.
"""
import math
from contextlib import ExitStack

import numpy as np
import ml_dtypes

import concourse.bass as bass
import concourse.bacc as bacc
import concourse.tile as tile
from concourse import mybir
from concourse.bass_utils import run_bass_kernel_spmd

F32 = mybir.dt.float32
F16 = mybir.dt.float16
BF16 = mybir.dt.bfloat16

B, S, D = 2, 2048, 2048
H, KV, HD, REP = 16, 4, 128, 4
NH = 4                     # heads per core
NW = S // 512              # q-windows
ND = D // 128              # d_in tiles
NU = S // 128              # kv tiles
SCALE = 1.0 / math.sqrt(HD)

# per-head-slot exp chunk width and bias recentering constant; slot 0 holds the
# steepest ALiBi slope of the core's group (heads are slope-descending within a
# group), so it gets the finest recentering. Range check (worst slope s per
# slot, scores*scale ~ N(0,1), P in bf16 with fp32-range exponent):
#   exp arg in [score - s*C, score + s*(W-1-C)] -> within e^{+-87} for
#   (W,C,s) = (128,96,.707), (256,127,.5), (256,127,.354), (512,255,.25).
EXP_W = [128, 256, 256, 512]
EXP_C = [96.0, 127.0, 127.0, 255.0]
NBM = 19                   # bias cols per head slot: m = gridpos - kvtile in [-3, 15]
# rope-pair swap within each 32-partition quadrant (pairs live at +0/+16)
SHUF_MASK = list(range(16, 32)) + list(range(16))


def build():
    nc = bacc.Bacc(None)
    xT_d = nc.dram_tensor("xT", [D, S], F16, kind="ExternalInput")
    # weights host-packed as [128, ND, out]: DMA lines are contiguous per
    # partition row (2KB+ for wq), full DMA throughput from the first chunk
    wq_d = nc.dram_tensor("wqT", [128, ND, NH * HD], F16, kind="ExternalInput")
    wk_d = nc.dram_tensor("wkT", [128, ND, HD], F16, kind="ExternalInput")
    wv_d = nc.dram_tensor("wvT", [128, ND, HD], F16, kind="ExternalInput")
    wo_d = nc.dram_tensor("woT", [NH * HD, D], F16, kind="ExternalInput")
    cosF_d = nc.dram_tensor("cosF", [128, S], F16, kind="ExternalInput")
    sinF_d = nc.dram_tensor("sinF", [128, S], F16, kind="ExternalInput")
    biasb_d = nc.dram_tensor("biasb", [128, NH * NBM], F32, kind="ExternalInput")
    cmask_d = nc.dram_tensor("cmask", [128, 128], BF16, kind="ExternalInput")
    ident_d = nc.dram_tensor("ident", [128, 128], BF16, kind="ExternalInput")
    ones_d = nc.dram_tensor("ones", [128, 128], BF16, kind="ExternalInput")
    part_d = nc.dram_tensor("part", [S, D], F16, kind="ExternalOutput")

    PSUM = bass.MemorySpace.PSUM

    with tile.TileContext(nc) as tc:
        with ExitStack() as ctx:
            consts = ctx.enter_context(tc.tile_pool(name="consts", bufs=1))
            persist = ctx.enter_context(tc.tile_pool(name="persist", bufs=1))

            cosF = consts.tile([128, S], F16, tag="cosF")
            sinF = consts.tile([128, S], F16, tag="sinF")
            biasb = consts.tile([128, NH * NBM], F32, tag="biasb")
            cmask = consts.tile([128, 128], BF16, tag="cmask")
            ident = consts.tile([128, 128], BF16, tag="ident")
            ones = consts.tile([128, 128], BF16, tag="ones")

            qT = [persist.tile([128, S], F16, tag=f"qT{h}", name=f"qT{h}")
                  for h in range(NH)]
            kT = persist.tile([128, S], F16, tag="kT")
            vnat = persist.tile([128, S], BF16, tag="vnat")
            attn = [persist.tile([128, S], F16, tag=f"attn{h}", name=f"attn{h}")
                    for h in range(NH)]

            # ---------------- phase 1: Q/K/V projections (+ per-window RoPE) -----
            with tc.tile_pool(name="wqkv", bufs=1) as wpool, \
                 tc.tile_pool(name="xsl", bufs=14) as xpool, \
                 tc.tile_pool(name="vtmp", bufs=1) as vpool, \
                 tc.tile_pool(name="rope", bufs=3) as rp, \
                 tc.tile_pool(name="pps", bufs=1, space=PSUM) as pps, \
                 tc.tile_pool(name="tpp", bufs=1, space=PSUM) as tpp:
                wq_sb = wpool.tile([128, ND, NH * HD], F16, tag="wq")
                wk_sb = wpool.tile([128, ND, HD], F16, tag="wk")
                wv_sb = wpool.tile([128, ND, HD], F16, tag="wv")

                xtiles = {}

                def issue_x(w, d):
                    xs = xpool.tile([128, 512], F16, tag="x", name="xs")
                    nc.sync.dma_start(
                        xs[:], xT_d[d * 128:(d + 1) * 128, w * 512:(w + 1) * 512])
                    xtiles[(w, d)] = xs

                # critical path on the sync hwdge queue (first to issue):
                # the small wk/wv head chunks, then window-0 x tiles;
                # remaining weight chunks stream JIT from gpsimd, followed by
                # the rope tables/consts (first needed ~25us in)
                d01 = slice(0, 2)
                nc.sync.dma_start(wk_sb[:, d01, :], wk_d[:, d01, :])
                nc.sync.dma_start(wv_sb[:, d01, :], wv_d[:, d01, :])
                issue_x(0, 0)
                nc.sync.dma_start(wq_sb[:, 0:1, :], wq_d[:, 0:1, :])
                nc.sync.dma_start(wq_sb[:, 1:2, :], wq_d[:, 1:2, :])
                for g2 in range(1, ND // 2):
                    dsl = slice(2 * g2, 2 * g2 + 2)
                    nc.gpsimd.dma_start(wk_sb[:, dsl, :], wk_d[:, dsl, :])
                    nc.gpsimd.dma_start(wv_sb[:, dsl, :], wv_d[:, dsl, :])
                    nc.gpsimd.dma_start(wq_sb[:, dsl, :], wq_d[:, dsl, :])
                for d in range(1, ND):
                    issue_x(0, d)
                nc.gpsimd.dma_start(cosF[:], cosF_d[:])
                nc.gpsimd.dma_start(sinF[:], sinF_d[:])
                nc.gpsimd.dma_start(biasb[:], biasb_d[:])
                nc.gpsimd.dma_start(cmask[:], cmask_d[:])
                nc.gpsimd.dma_start(ident[:], ident_d[:])
                nc.gpsimd.dma_start(ones[:], ones_d[:])
                vT = vpool.tile([128, S], BF16, tag="vT")

                for w in range(NW):
                    sl = slice(w * 512, (w + 1) * 512)
                    pq = [pps.tile([128, 512], F32, tag=f"pq{h}", name=f"pq{h}")
                          for h in range(NH)]
                    pk = pps.tile([128, 512], F32, tag="pk", name="pk")
                    pv = pps.tile([128, 512], F32, tag="pv", name="pv")
                    for d in range(ND):
                        if w < NW - 1:
                            issue_x(w + 1, d)   # one-window lookahead
                        xs = xtiles.pop((w, d))
                        st, sp = (d == 0), (d == ND - 1)
                        nc.tensor.matmul(pk[:], wk_sb[:, d, :], xs[:], start=st, stop=sp)
                        nc.tensor.matmul(pv[:], wv_sb[:, d, :], xs[:], start=st, stop=sp)
                        for h in range(NH):
                            nc.tensor.matmul(pq[h][:], wq_sb[:, d, h * 128:(h + 1) * 128],
                                             xs[:], start=st, stop=sp)
                    # PSUM->SBUF copies split across ACT and DVE
                    nc.scalar.copy(kT[:, sl], pk[:])
                    nc.scalar.copy(qT[0][:, sl], pq[0][:])
                    nc.scalar.copy(qT[1][:, sl], pq[1][:])
                    nc.vector.tensor_copy(vT[:, sl], pv[:])
                    nc.vector.tensor_copy(qT[2][:, sl], pq[2][:])
                    nc.vector.tensor_copy(qT[3][:, sl], pq[3][:])

                    # RoPE on this window's q/k slices: out = cosF*z +
                    # sinF*swap(z); rope pairs are laid out within 32-partition
                    # quadrants (host perm) so the swap is a DVE stream_shuffle.
                    # The last window's rope is deferred into phase 2 (its
                    # consumers are the w=3 attention units).
                    if w < NW - 1:
                        for tgt in [kT] + qT:
                            qb = rp.tile([128, 512], F16, tag="qb", name="qb")
                            nc.vector.stream_shuffle(qb[:], tgt[:, sl], SHUF_MASK)
                            t1 = rp.tile([128, 512], F16, tag="t1", name="t1")
                            nc.vector.tensor_mul(t1[:], tgt[:, sl], cosF[:, sl])
                            nc.vector.tensor_mul(qb[:], qb[:], sinF[:, sl])
                            nc.vector.tensor_add(tgt[:, sl], t1[:], qb[:])

                    # V transpose for this window's 4 kv tiles:
                    # vT [hd, s] -> vnat [s(part), hd]
                    for u in range(4 * w, 4 * w + 4):
                        tp = tpp.tile([128, 128], BF16, tag=f"tp{u % 2}",
                                      name=f"tp{u}")
                        nc.tensor.transpose(tp[:], vT[:, u * 128:(u + 1) * 128],
                                            ident[:])
                        nc.scalar.copy(vnat[:, u * 128:(u + 1) * 128], tp[:])

            # ---------------- phase 2: attention + output projection ------------
            with tc.tile_pool(name="sp", bufs=3, space=PSUM) as sp, \
                 tc.tile_pool(name="dp", bufs=1, space=PSUM) as dp, \
                 tc.tile_pool(name="op", bufs=2, space=PSUM) as op, \
                 tc.tile_pool(name="ojp", bufs=2, space=PSUM) as ojp, \
                 tc.tile_pool(name="Pp", bufs=8) as Pp, \
                 tc.tile_pool(name="psum_sb", bufs=3) as psb, \
                 tc.tile_pool(name="rope2", bufs=2) as rp2, \
                 tc.tile_pool(name="ep", bufs=4) as ep, \
                 tc.tile_pool(name="wop", bufs=1) as wop, \
                 tc.tile_pool(name="ostg", bufs=6) as ostg:
                wo_sb = wop.tile([128, NH, D], F16, tag="wo")
                nc.gpsimd.dma_start(wo_sb[:], wo_d.rearrange("(h p) o -> p h o", p=128))

                # deferred rope for the last window, spread one tensor per
                # early phase-2 unit (consumers are the w=3 units much later)
                sl3 = slice((NW - 1) * 512, NW * 512)
                rope_q = [qT[3], qT[2], qT[1], qT[0], kT]

                def emit_rope():
                    if not rope_q:
                        return
                    tgt = rope_q.pop(0)
                    qb = rp2.tile([128, 512], F16, tag="qb2", name="qb2")
                    nc.vector.stream_shuffle(qb[:], tgt[:, sl3], SHUF_MASK)
                    t1 = rp2.tile([128, 512], F16, tag="t12", name="t12")
                    nc.vector.tensor_mul(t1[:], tgt[:, sl3], cosF[:, sl3])
                    nc.vector.tensor_mul(qb[:], qb[:], sinF[:, sl3])
                    nc.vector.tensor_add(tgt[:, sl3], t1[:], qb[:])

                filler_q = []

                def emit_fillers(n, eng="v", queue=None):
                    for _ in range(n):
                        if not filler_q:
                            return
                        filler_q.pop(0)(eng, queue)

                def make_unit(w_, mq_, dwin_):
                    def unit(eng, queue):
                        m_ = 4 * w_ + mq_
                        po = ojp.tile([128, 512], F32, tag="oj",
                                      name=f"po{m_}_{dwin_}")
                        for h_ in range(NH):
                            nc.tensor.matmul(
                                po[:],
                                attn[h_][:, m_ * 128:(m_ + 1) * 128],
                                wo_sb[:, h_, dwin_ * 512:(dwin_ + 1) * 512],
                                start=(h_ == 0), stop=(h_ == NH - 1))
                        so = ostg.tile([128, 512], F16, tag="so", name="so")
                        if eng == "v":
                            nc.vector.tensor_copy(so[:], po[:])
                        else:
                            nc.scalar.copy(so[:], po[:])
                        dq_eng = queue if queue is not None else nc.sync
                        dq_eng.dma_start(
                            part_d[m_ * 128:(m_ + 1) * 128,
                                   dwin_ * 512:(dwin_ + 1) * 512], so[:])
                    return unit

                ucount = 0
                # per-window filler emission caps: the late windows are
                # ACT/DVE-paced with PE slack, the early ones are PE-paced;
                # carrying units from w=1 into w=2/3 matches filler supply
                # to where the PE actually idles (trace: filler_q exhausted
                # halfway through w=2/3 leaving ~0.5us gaps every 2 u-steps)
                emit_cap = {0: 0, 1: 8, 2: 16, 3: 22}
                for w in range(NW):
                    emitted_in_w = 0
                    qsl = slice(w * 512, (w + 1) * 512)
                    U = 4 * (w + 1)
                    # window 0 runs cheapest-exp heads first (no filler supply
                    # yet, so let ACT race ahead of the PE)
                    horder = [3, 2, 1, 0] if w == 0 else [0, 1, 2, 3]
                    for h in horder:
                        o_ps = op.tile([128, 512], F32, tag="o", name=f"o{w}_{h}")
                        d_ps = dp.tile([128, 512], F32, tag="den", name=f"d{w}_{h}")
                        # hybrid softmax denominator: for w>=1 the early full
                        # tiles (u <= U-4, all n0=0) accumulate elementwise
                        # into a bf16 partial sum on the DVE, reduced by ONE
                        # ones^T matmul; only the last 3 diagonal tiles keep
                        # the per-tile PE matmul path (their adds would land
                        # too late to hide). This removes ~2/3 of the den's
                        # PE column traffic. w=0 units are tiny: all-PE.
                        chain = w >= 1
                        psA = psb.tile([128, 512], BF16, tag="psA",
                                       name="psA") if chain else None
                        # PV runs LAG tiles behind the scores/exp front so
                        # each exp has a full extra iteration to land before
                        # the PE consumes its P tile (the scores->exp->PV
                        # round trip was showing ~0.5us PE waits per 2 steps
                        # in the late windows)
                        lag = 1
                        pendq = []
                        dq = []
                        for u in range(U):
                            i0 = max(0, u - 4 * w)
                            n0 = 128 * i0
                            s_ps = sp.tile([128, 512], F32, tag="s", name="s")
                            nc.tensor.matmul(
                                s_ps[:, n0:512],
                                kT[:, u * 128:(u + 1) * 128],
                                qT[h][:, w * 512 + n0:(w + 1) * 512],
                                start=True, stop=True)
                            if chain and u == U - 1:
                                # partial-sum reduce; its last add (u=U-4)
                                # finished iterations ago, so no PE stall
                                nc.tensor.matmul(d_ps[:], ones[:], psA[:],
                                                 start=True, stop=False)
                            Pt = Pp.tile([128, 512], BF16, tag="P", name="P")
                            # exp pieces on a fixed per-slot q-grid (grid
                            # position independent of u so the dropped per-q
                            # ALiBi term is consistent across kv tiles);
                            # diagonal tiles clip the first piece at n0.
                            # bias col m = grid position (128-units) - kv tile
                            for gs in range(0, 512, EXP_W[h]):
                                pcs = max(gs, n0)
                                pce = gs + EXP_W[h]
                                if pcs >= pce:
                                    continue
                                m = 4 * w + gs // 128 - u + 3
                                nc.scalar.activation(
                                    Pt[:, pcs:pce],
                                    s_ps[:, pcs:pce],
                                    mybir.ActivationFunctionType.Exp,
                                    bias=biasb[:, h * NBM + m:h * NBM + m + 1],
                                    scale=SCALE)
                            if u >= 4 * w:
                                # diagonal kv tile: causal 0/1 mask on its block
                                nc.vector.tensor_mul(Pt[:, n0:n0 + 128],
                                                     Pt[:, n0:n0 + 128], cmask[:])
                            if chain and u <= U - 4:
                                if u == 0:
                                    nc.vector.tensor_copy(psA[:], Pt[:])
                                else:
                                    nc.vector.tensor_add(psA[:], psA[:], Pt[:])
                            ucount += 1
                            # front-load PE filler onto the exp-heavy slot-0
                            # head; later slots are self-paced. Hold fillers
                            # for the first u-steps of each window's first
                            # head: they read the PREVIOUS window's attn whose
                            # recip/mul chain is still draining on DVE, and
                            # they would block the new window's scores in the
                            # in-order PE queue.
                            if ((h == 0 and u >= 2)
                                    or (h != 0 and ucount % 2 == 0)) \
                                    and emitted_in_w < emit_cap[w]:
                                emit_fillers(1)
                                emitted_in_w += 1
                            pendq.append((Pt, n0, u))
                            if len(pendq) > lag:
                                pPt, pn0, pu = pendq.pop(0)
                                nc.tensor.matmul(o_ps[:, pn0:512],
                                                 vnat[:, pu * 128:(pu + 1) * 128],
                                                 pPt[:, pn0:512],
                                                 start=(pu == 0), stop=False)
                                if not chain or pu >= U - 3:
                                    dq.append((pPt, pn0, pu))
                                if len(dq) > 2 and not chain:
                                    dPt, dn0, du = dq.pop(0)
                                    nc.tensor.matmul(d_ps[:, dn0:512], ones[:],
                                                     dPt[:, dn0:512],
                                                     start=(du == 0), stop=False)
                        for pPt, pn0, pu in pendq:
                            nc.tensor.matmul(o_ps[:, pn0:512],
                                             vnat[:, pu * 128:(pu + 1) * 128],
                                             pPt[:, pn0:512],
                                             start=(pu == 0), stop=(pu == U - 1))
                            if not chain or pu >= U - 3:
                                dq.append((pPt, pn0, pu))
                        pendq = []
                        for qi, (dPt, dn0, du) in enumerate(dq):
                            nc.tensor.matmul(d_ps[:, dn0:512], ones[:],
                                             dPt[:, dn0:512],
                                             start=(False if chain else du == 0),
                                             stop=(qi == len(dq) - 1))
                        dq = []
                        rec = ep.tile([128, 512], F32, tag="rec", name="rec")
                        nc.vector.reciprocal_approx_fast(rec[:], d_ps[:])
                        nc.vector.tensor_mul(attn[h][:, qsl], o_ps[:], rec[:])
                        # deferred window-3 rope rides the early-unit DVE slack
                        if w <= 1:
                            emit_rope()

                    # enqueue this window's output projection as PE filler
                    # for the next window's attention loop
                    for mq in range(4):
                        for dwin in range(4):
                            filler_q.append(make_unit(w, mq, dwin))

                # drain remaining units; staging copies alternate DVE/ACT and
                # the output DMAs rotate over four queues (all near-idle now)
                di = 0
                queues = [nc.sync, nc.gpsimd, nc.scalar]
                while filler_q:
                    eng = "v" if di % 2 == 0 else "s"
                    emit_fillers(1, eng=eng, queue=queues[di % 3])
                    di += 1
    nc.finalize()
    return nc


_NC_CACHE = {}


def _get_nc():
    if "nc" not in _NC_CACHE:
        _NC_CACHE["nc"] = build()
    return _NC_CACHE["nc"]


def _host_prep(x, alibi_bias, wq, wk, wv, wo):
    """Build per-core input maps (shard + transpose + rope tables + bias tables)."""
    x = np.asarray(x, np.float32)
    alibi_bias = np.asarray(alibi_bias, np.float32)
    wq = np.asarray(wq, np.float32)
    wk = np.asarray(wk, np.float32)
    wv = np.asarray(wv, np.float32)
    wo = np.asarray(wo, np.float32)
    bf16 = ml_dtypes.bfloat16

    slopes = alibi_bias[0, :, 0, 1].copy()        # [H]; alibi[0,h,0,1] = slope_h

    inv_freq = 1.0 / (10000.0 ** (np.arange(0, HD, 2, dtype=np.float32) / HD))
    t = np.arange(S, dtype=np.float32)
    freqs = np.outer(t, inv_freq)                 # [S, 64]
    cos = np.cos(freqs).astype(np.float32).T      # [64, S]
    sin = np.sin(freqs).astype(np.float32).T
    # quadrant-paired rope layout: pair i -> quadrant i//16, slots j and j+16
    # (even element at row 32*(i//16)+i%16, odd at +16); the on-device swap is
    # then a within-quadrant stream_shuffle
    perm = np.zeros(HD, np.int64)
    row_f = np.zeros(HD, np.int64)    # freq index per row
    row_sg = np.zeros(HD, np.float32)  # sin sign per row
    for i in range(64):
        qd, j = divmod(i, 16)
        perm[32 * qd + j] = 2 * i
        perm[32 * qd + 16 + j] = 2 * i + 1
        row_f[32 * qd + j] = i
        row_f[32 * qd + 16 + j] = i
        row_sg[32 * qd + j] = -1.0
        row_sg[32 * qd + 16 + j] = 1.0
    cosF = np.ascontiguousarray(cos[row_f]).astype(np.float16)
    sinF = np.ascontiguousarray(sin[row_f] * row_sg[:, None]).astype(np.float16)
    p_ar = np.arange(128, dtype=np.float32)
    cmask = (p_ar[:, None] <= p_ar[None, :]).astype(bf16)
    ident = np.eye(128, dtype=np.float32).astype(bf16)
    ones = np.ones((128, 128), np.float32).astype(bf16)

    def pack_ptd(wT):
        # [D, out] -> [128, ND, out] so SBUF-layout DMA lines are contiguous
        return np.ascontiguousarray(
            wT.reshape(ND, 128, wT.shape[1]).transpose(1, 0, 2))

    xTs = [np.ascontiguousarray(x[b].T).astype(np.float16) for b in range(B)]
    in_maps = []
    for core in range(8):
        b, g = divmod(core, KV)
        wq_g = wq[4 * g * HD:(4 * g + 4) * HD].reshape(NH, HD, D)[:, perm, :]
        wqT = np.ascontiguousarray(wq_g.reshape(NH * HD, D).T).astype(np.float16)
        wkT = np.ascontiguousarray(wk[g * HD:(g + 1) * HD][perm].T).astype(np.float16)
        wvT = np.ascontiguousarray(wv[g * HD:(g + 1) * HD].T).astype(np.float16)
        woT = np.ascontiguousarray(
            wo[:, 4 * g * HD:(4 * g + 4) * HD].T).astype(np.float16)
        biasb = np.zeros((128, NH * NBM), np.float32)
        for h in range(NH):
            sl = slopes[4 * g + h]
            for mi in range(NBM):
                m = mi - 3   # m = gridpos - kvtile; bias = slope*(j - i_gs - C)
                biasb[:, h * NBM + mi] = sl * (p_ar - 128.0 * m - EXP_C[h])
        in_maps.append({
            "xT": xTs[b], "wqT": pack_ptd(wqT), "wkT": pack_ptd(wkT),
            "wvT": pack_ptd(wvT), "woT": woT,
            "cosF": cosF, "sinF": sinF, "biasb": biasb, "cmask": cmask,
            "ident": ident, "ones": ones,
        })
    return in_maps


def kernel(x, mask, alibi_bias, wq, wk, wv, wo, _trace=False, _trace_kwargs=None):
    nc = _get_nc()
    in_maps = _host_prep(x, alibi_bias, wq, wk, wv, wo)
    res = run_bass_kernel_spmd(nc, in_maps, list(range(8)), trace=_trace,
                               **(_trace_kwargs or {}))
    parts = [np.asarray(res.results[c]["part"], np.float32) for c in range(8)]
    out = np.stack([
        parts[0] + parts[1] + parts[2] + parts[3],
        parts[4] + parts[5] + parts[6] + parts[7],
    ]).astype(np.float32)
    if _trace:
        return out, res
    return out
